# revision 1
# baseline (speedup 1.0000x reference)
"""Trainium2 Bass kernel for the EnhancedMathematicalReasoning MoE-routing module.

Computation (per token t, hidden dim H=2048, E=8 experts, dense routing):
    a1     = gelu(h @ Wd1 + bd1)
    logits = a1 @ Wd2 + bd2
    op_w   = softmax(logits)
    comb   = sum_e op_w[:, e] * (h @ We[e] + be[e])
    out    = (gelu(comb @ Wi1 + bi1) @ Wi2 + bi2) * mask

Sharding: data-parallel over the 8192 tokens -> 1024 tokens per NeuronCore,
weights replicated, no collectives.

Per-core layout strategy (P=128):
  - h is PE-transposed once to hT [H, T] so H sits on partitions for every
    GEMM contraction.  All big GEMMs run in float32r (TF32 datapath - full PE
    rate at moving free dim >= 256, ~1e-4 relative accuracy).
  - GEMM1/GEMM3/GEMM4 produce [H_out, T] with the weight m-chunk stationary
    and the resident activation as a 512-wide moving operand.
  - Expert GEMMs run in [T, H_out] orientation (hT slices stationary, We
    chunks moving at N=256) so op_w[t, e] is a per-partition scalar and the
    weighted combine is one fused DVE op per psum eviction:
        comb = psum * op_w[:, e] + comb
  - All PE transposes are batched 4-to-a-PSUM-bank with a single batched
    eviction that alternates between the Scalar and Vector engines, keeping
    the PE fed during transpose phases.
  - comb -> combT transposes are interleaved into the last expert's tail;
    the final out-transpose + mask + store are interleaved into GEMM4.
"""

import numpy as np
from contextlib import ExitStack

import concourse.bass as bass
import concourse.tile as tile
from concourse import bacc, mybir
from concourse.bass_utils import run_bass_kernel_spmd
from concourse.masks import make_identity

F32 = mybir.dt.float32
F32R = mybir.dt.float32r
AF = mybir.ActivationFunctionType
ALU = mybir.AluOpType
AX = mybir.AxisListType

P = 128
N_CORES = 8

B, S, H_FULL, E_FULL = 4, 2048, 2048, 8


def build_nc(T, H, E, act=AF.Gelu, include_be=False):
    """Build + compile the single-core program (same program runs SPMD on all
    cores). T: tokens per core. Requires T % 512 == 0, H % 512 == 0."""
    assert T % 512 == 0 and H % 512 == 0 and E <= P
    KT = H // P
    TT = T // P
    TB = T // 512
    NB = H // 256
    MT = H // P

    nc = bacc.Bacc("TRN2", target_bir_lowering=False, debug=False)

    h_d = nc.dram_tensor("h", [T, H], F32R, kind="ExternalInput").ap()
    msk_d = nc.dram_tensor("mask", [T], F32, kind="ExternalInput").ap()
    wd1_d = nc.dram_tensor("wd1", [H, H], F32R, kind="ExternalInput").ap()
    bd1_d = nc.dram_tensor("bd1", [H], F32, kind="ExternalInput").ap()
    wd2_d = nc.dram_tensor("wd2", [H, E], F32R, kind="ExternalInput").ap()
    bd2_d = nc.dram_tensor("bd2", [E], F32, kind="ExternalInput").ap()
    we_d = nc.dram_tensor("we", [E, H, H], F32R, kind="ExternalInput").ap()
    be_d = nc.dram_tensor("be", [E, H], F32R, kind="ExternalInput").ap()
    wi1_d = nc.dram_tensor("wi1", [H, H], F32R, kind="ExternalInput").ap()
    bi1_d = nc.dram_tensor("bi1", [H], F32, kind="ExternalInput").ap()
    wi2_d = nc.dram_tensor("wi2", [H, H], F32R, kind="ExternalInput").ap()
    bi2_d = nc.dram_tensor("bi2", [H], F32, kind="ExternalInput").ap()
    out_d = nc.dram_tensor("out", [T, H], F32, kind="ExternalOutput").ap()

    wd1_re = wd1_d.rearrange("(k p) n -> p k n", p=P)
    wi1_re = wi1_d.rearrange("(k p) n -> p k n", p=P)
    wi2_re = wi2_d.rearrange("(k p) n -> p k n", p=P)

    with tile.TileContext(nc) as tc:
        with ExitStack() as ctx:
            const = ctx.enter_context(tc.tile_pool(name="const", bufs=1))
            bigp = ctx.enter_context(tc.tile_pool(name="bigp", bufs=1))
            wep = ctx.enter_context(tc.tile_pool(name="wep", bufs=2))
            scr = ctx.enter_context(tc.tile_pool(name="scr", bufs=2))
            osm = ctx.enter_context(tc.tile_pool(name="osm", bufs=2))
            opb = ctx.enter_context(tc.tile_pool(name="opb", bufs=3))
            smp = ctx.enter_context(tc.tile_pool(name="smp", bufs=4))
            pp = ctx.enter_context(tc.tile_pool(name="pp", bufs=2, space="PSUM"))

            # ---- constants (engine-generated; no DMA) ----
            ident = const.tile([P, P], F32, name="ident")
            make_identity(nc, ident)
            ones1 = const.tile([1, P], F32, name="ones1")
            nc.vector.memset(ones1, 1.0)
            identR = const.tile([P, P], F32R, name="identR")
            nc.scalar.copy(identR, ident)
            # sel8[e', e*128+p] = (e' == e): K=8 selector used to broadcast
            # op_w rows across all 128 partitions via a tiny matmul.
            sel8f = const.tile([E, E, P], F32, name="sel8f")
            nc.gpsimd.memset(sel8f, 0.0)
            nc.gpsimd.affine_select(
                out=sel8f, in_=sel8f, compare_op=ALU.not_equal, fill=1.0,
                base=0, pattern=[[-1, E], [0, P]], channel_multiplier=1)
            sel8 = const.tile([E, E * P], F32R, name="sel8")
            nc.scalar.copy(sel8, sel8f.rearrange("e a p -> e (a p)"))
            opwT = const.tile([E, T], F32R, name="opwT")
            opw = const.tile([P, TT, E], F32, name="opw")
            lacc = const.tile([P, TT, E], F32, name="lacc")

            # Batched PE transpose: up to 4 [128,128] tiles share one PSUM bank
            # (one accumulation group, disjoint column writes), then a single
            # batched eviction on an alternating engine keeps ACT/DVE balanced.
            ecnt = [0]

            def tbatch(srcs, out3, scale=None, dma=None):
                n = len(srcs)
                trp = pp.tile([P, 4, P], F32R, tag="tr", bufs=3, name="trb")
                for i, s in enumerate(srcs):
                    nc.tensor.matmul(trp[:, i, :], s, identR, is_transpose=True,
                                     start=(i == 0), stop=(i == n - 1))
                src = trp[:, :n, :]
                ecnt[0] += 1
                if dma is not None:
                    ot = osm.tile([P, 4, P], F32, tag="os", name="ot")
                    dst = ot[:, :n, :]
                else:
                    dst = out3
                if scale is None:
                    if ecnt[0] % 2 == 0:
                        nc.scalar.copy(dst, src)
                    else:
                        nc.vector.tensor_copy(dst, src)
                else:
                    if ecnt[0] % 2 == 0:
                        nc.scalar.activation(dst, src, AF.Copy, scale=scale)
                    else:
                        nc.vector.tensor_scalar_mul(dst, src, scale)
                if dma is not None:
                    nc.sync.dma_start(dma, dst)

            # ---- stage A: load h, transpose to hT [H, T] ----
            hT = bigp.tile([P, KT, T], F32R, tag="A", name="hT")
            w1_0 = None
            ttorder = list(range(TT))
            ttorder.insert(min(4, TT), -1)
            for tt in ttorder:
                if tt == -1:
                    # GEMM1's first weight chunk: issued here so its DMA is not
                    # queued behind the tt4..7 h-loads (12us PE gap otherwise).
                    w1_0 = wep.tile([P, KT, 256], F32R, tag="we", name="wd1m_0")
                    nc.sync.dma_start(w1_0, wd1_re[:, :, 0:256])
                    continue
                for half in range(2):
                    hw = min(1024, H - half * 1024)
                    if hw <= 0:
                        continue
                    hl = scr.tile([P, 1024], F32R, tag="s", bufs=3, name=f"hl_{tt}_{half}")
                    nc.sync.dma_start(
                        hl[:, :hw],
                        h_d[tt * P:(tt + 1) * P, half * 1024:half * 1024 + hw])
                    for kg in range(hw // P // 4):
                        k0 = half * 8 + kg * 4
                        tbatch(
                            [hl[:, (kg * 4 + j) * P:(kg * 4 + j + 1) * P]
                             for j in range(4)],
                            hT[:, k0:k0 + 4, tt * P:(tt + 1) * P])

            # ---- constant DMA loads (emitted after stage A so the h
            # loads win the DMA queues at kernel start) ----
            wd2_t = const.tile([P, KT, E], F32R, name="wd2_t")
            nc.sync.dma_start(wd2_t, wd2_d.rearrange("(k p) e -> p k e", p=P))
            bd1_t = const.tile([P, KT], F32, name="bd1_t")
            nc.sync.dma_start(bd1_t, bd1_d.rearrange("(k p) -> p k", p=P))
            bi1_t = const.tile([P, KT], F32, name="bi1_t")
            nc.sync.dma_start(bi1_t, bi1_d.rearrange("(k p) -> p k", p=P))
            bi2_t = const.tile([P, KT], F32, name="bi2_t")
            nc.sync.dma_start(bi2_t, bi2_d.rearrange("(k p) -> p k", p=P))
            bd2_t = const.tile([1, E], F32, name="bd2_t")
            nc.sync.dma_start(bd2_t, bd2_d.unsqueeze(0))
            mask_t = const.tile([P, TT], F32, name="mask_t")
            nc.sync.dma_start(mask_t, msk_d.rearrange("(t p) -> p t", p=P))
            if include_be:
                be_t = const.tile([E, H], F32R, name="be_t")
                nc.sync.dma_start(be_t, be_d)

            # ---- stage B: a1T = act(Wd1.T @ hT + bd1) + fused logits GEMM ----
            for mg in range(MT // 2):
                if mg == 0 and w1_0 is not None:
                    w1 = w1_0
                else:
                    w1 = wep.tile([P, KT, 256], F32R, tag="we", name=f"wd1m_{mg}")
                    nc.sync.dma_start(w1, wd1_re[:, :, mg * 256:(mg + 1) * 256])
                for mi in range(2):
                    m = 2 * mg + mi
                    a1 = scr.tile([P, T], F32R, tag="s", bufs=3, name=f"a1_{m}")
                    for tb in range(TB):
                        ps = pp.tile([P, 512], F32, tag="mm", bufs=3, name="ps_g1")
                        for k in range(KT):
                            nc.tensor.matmul(ps, w1[:, k, mi * P:(mi + 1) * P],
                                             hT[:, k, tb * 512:(tb + 1) * 512],
                                             start=(k == 0), stop=(k == KT - 1))
                        nc.scalar.activation(a1[:, tb * 512:(tb + 1) * 512], ps,
                                             act, bias=bd1_t[:, m:m + 1])
                    for tt in range(TT):
                        lg = pp.tile([P, E], F32, tag="lgt", bufs=1, name="lg")
                        nc.tensor.matmul(lg, a1[:, tt * P:(tt + 1) * P],
                                         wd2_t[:, m, :], start=True, stop=(m > 0))
                        if m == 0:
                            nc.tensor.matmul(lg, ones1, bd2_t, start=False,
                                             stop=True)
                            nc.vector.tensor_copy(lacc[:, tt, :], lg)
                        else:
                            nc.vector.tensor_add(lacc[:, tt, :], lacc[:, tt, :], lg)

            # ---- softmax over E ----
            for tt in range(TT):
                nmax = smp.tile([P, 1], F32, tag="sm1", bufs=6, name="nmax")
                nc.vector.reduce_max(nmax, lacc[:, tt, :], AX.X, negate=True)
                et = smp.tile([P, E], F32, tag="sme", bufs=2, name="et")
                nc.scalar.activation(et, lacc[:, tt, :], AF.Exp, bias=nmax,
                                     scale=1.0)
                ssum = smp.tile([P, 1], F32, tag="sm1", bufs=6, name="ssum")
                nc.vector.reduce_sum(ssum, et, AX.X)
                rin = smp.tile([P, 1], F32, tag="sm1", bufs=6, name="rin")
                nc.vector.reciprocal(rin, ssum)
                nc.vector.tensor_scalar_mul(opw[:, tt, :], et, rin)
            # Hoisted: the first expert chunk's psum fills depend only on hT
            # and We[0], so they keep the PE busy while the serial softmax ->
            # op_w-transpose -> broadcast chain resolves on DVE/ACT.
            wet00 = wep.tile([P, KT, 256], F32R, tag="we", name="we_0_0")
            nc.sync.dma_start(
                wet00,
                we_d[0].rearrange("(k p) n -> p k n", p=P)[:, :, 0:256])
            hoist_ps = []
            for tb in range(TB):
                ps = pp.tile([P, 512], F32, tag="mm", bufs=3, name="eps_h")
                for k in range(KT):
                    nc.tensor.matmul(ps, wet00[:, k, 0:P],
                                     hT[:, k, tb * 512:(tb + 1) * 512],
                                     start=(k == 0), stop=(k == KT - 1))
                hoist_ps.append(ps)
            for tt in range(TT):
                trp = pp.tile([P, 4, P], F32, tag="tr", bufs=3, name="trp_ow")
                nc.tensor.matmul(trp[:E, 0, :], opw[:, tt, :], ident,
                                 is_transpose=True, start=True, stop=True)
                nc.scalar.copy(opwT[:, tt * P:(tt + 1) * P], trp[:E, 0, :])

            # ---- stage C: expert GEMMs in [H_out, T] orientation.
            # Stationary = We m-chunk, moving = resident hT at N=512 (full
            # fp32r rate, ~227ns per 512-row MM).  op_w[t, e] is broadcast
            # across partitions as opb = sel8[:, e].T @ opwT (a K=8 matmul),
            # and the weighted combine accumulates straight into combT [H, T]:
            #     combT[m, t] += opb[t] * psum[m, t]
            # eliminating the [T,H] comb buffer and its 128 PE transposes.
            arena = bigp.tile([P, KT, TT, P], F32, tag="B", name="arena")
            arenaR = arena.bitcast(F32R)

            if include_be:
                # init combT with the op_w-weighted bias term:
                #   combT[m*128+p, t] = sum_e op_w[t, e] * be[e, m*128+p]
                for m in range(MT):
                    for tb in range(TB):
                        bps = pp.tile([P, 512], F32, tag="mm", bufs=3, name="bps")
                        nc.tensor.matmul(bps, be_t[:, m * P:(m + 1) * P],
                                         opwT[:, tb * 512:(tb + 1) * 512],
                                         start=True, stop=True)
                        nc.scalar.copy(
                            arenaR[:, m, tb * 4:(tb + 1) * 4, :],
                            bps.rearrange("p (n c) -> p n c", c=P))

            obs = {}
            for e in range(E):
                we_re = we_d[e].rearrange("(k p) n -> p k n", p=P)
                for tb in range(TB):
                    bps = pp.tile([P, 512], F32, tag="mm", bufs=3, name="bps")
                    nc.tensor.matmul(bps, sel8[:, e * P:(e + 1) * P],
                                     opwT[:, tb * 512:(tb + 1) * 512],
                                     start=True, stop=True)
                    ob = opb.tile([P, 512], F32, tag="ob", bufs=3,
                                  name=f"ob_{e}_{tb}")
                    nc.scalar.copy(ob, bps)
                    obs[tb] = ob
                for mg in range(MT // 2):
                    if e == 0 and mg == 0:
                        wet = wet00
                    else:
                        wet = wep.tile([P, KT, 256], F32R, tag="we",
                                       name=f"we_{e}_{mg}")
                        nc.sync.dma_start(wet,
                                          we_re[:, :, mg * 256:(mg + 1) * 256])
                    for mi in range(2):
                        m = 2 * mg + mi
                        for tb in range(TB):
                            if e == 0 and mg == 0 and mi == 0:
                                ps = hoist_ps[tb]
                            else:
                                ps = pp.tile([P, 512], F32, tag="mm", bufs=3,
                                             name="eps")
                                for k in range(KT):
                                    nc.tensor.matmul(
                                        ps, wet[:, k, mi * P:(mi + 1) * P],
                                        hT[:, k, tb * 512:(tb + 1) * 512],
                                        start=(k == 0), stop=(k == KT - 1))
                            wsl = arenaR[:, m, tb * 4:(tb + 1) * 4, :]
                            rsl = arena[:, m, tb * 4:(tb + 1) * 4, :]
                            ob3 = obs[tb].rearrange("p (n c) -> p n c", c=P)
                            ps3 = ps.rearrange("p (n c) -> p n c", c=P)
                            if e == 0 and not include_be:
                                nc.vector.tensor_tensor(wsl, ps3, ob3,
                                                        op=ALU.mult)
                            else:
                                tmp = scr.tile([P, 512], F32, tag="s",
                                               bufs=3, name="tmp")
                                tmp3 = tmp.rearrange("p (n c) -> p n c", c=P)
                                nc.vector.tensor_tensor(tmp3, ps3, ob3,
                                                        op=ALU.mult)
                                nc.vector.tensor_tensor(wsl, rsl, tmp3,
                                                        op=ALU.add)

            # ---- stage E: a2T = act(Wi1.T @ combT + bi1) ----
            a2T = bigp.tile([P, KT, T], F32R, tag="A", name="a2T")
            for mg in range(MT // 2):
                w3 = wep.tile([P, KT, 256], F32R, tag="we", name=f"wi1m_{mg}")
                nc.sync.dma_start(w3, wi1_re[:, :, mg * 256:(mg + 1) * 256])
                for mi in range(2):
                    m = 2 * mg + mi
                    for tb in range(TB):
                        ps = pp.tile([P, 512], F32, tag="mm", bufs=3, name="ps_g3")
                        for k in range(KT):
                            nc.tensor.matmul(ps, w3[:, k, mi * P:(mi + 1) * P],
                                             arenaR[:, k, tb * 4:(tb + 1) * 4, :],
                                             start=(k == 0), stop=(k == KT - 1))
                        nc.scalar.activation(a2T[:, m, tb * 512:(tb + 1) * 512],
                                             ps, act, bias=bi1_t[:, m:m + 1])

            # ---- stage F: outT = Wi2.T @ a2T + bi2, with the out-transpose,
            #      mask and store interleaved every 4 m-tiles ----
            outT = bigp.tile([P, KT, T], F32R, tag="B", name="outT")
            for mg in range(MT // 2):
                w4 = wep.tile([P, KT, 256], F32R, tag="we", name=f"wi2m_{mg}")
                nc.sync.dma_start(w4, wi2_re[:, :, mg * 256:(mg + 1) * 256])
                for mi in range(2):
                    m = 2 * mg + mi
                    for tb in range(TB):
                        ps = pp.tile([P, 512], F32, tag="mm", bufs=3, name="ps_g4")
                        for k in range(KT):
                            nc.tensor.matmul(ps, w4[:, k, mi * P:(mi + 1) * P],
                                             a2T[:, k, tb * 512:(tb + 1) * 512],
                                             start=(k == 0), stop=(k == KT - 1))
                        nc.scalar.activation(outT[:, m, tb * 512:(tb + 1) * 512],
                                             ps, AF.Identity,
                                             bias=bi2_t[:, m:m + 1])
                    if m % 4 == 3:
                        for tt in range(TT):
                            tbatch(
                                [outT[:, m - 3 + j, tt * P:(tt + 1) * P]
                                 for j in range(4)],
                                None,
                                scale=mask_t[:, tt:tt + 1],
                                dma=out_d[tt * P:(tt + 1) * P,
                                          (m - 3) * P:(m + 1) * P].rearrange(
                                              "t (n c) -> t n c", c=P))

    nc.compile()
    return nc


_CACHED = {}


def _get_nc(T, H, E, include_be):
    key = (T, H, E, include_be)
    if key not in _CACHED:
        _CACHED[key] = build_nc(T, H, E, act=AF.Gelu, include_be=include_be)
    return _CACHED[key]


def kernel(hidden_states, attention_mask, Wd1, bd1, Wd2, bd2, We, be, Wi1, bi1,
           Wi2, bi2, _trace=False):
    f32 = lambda x: np.ascontiguousarray(np.asarray(x, dtype=np.float32))
    h = f32(hidden_states)
    mask = f32(attention_mask)
    Wd1, bd1, Wd2, bd2 = f32(Wd1), f32(bd1), f32(Wd2), f32(bd2)
    We, be, Wi1, bi1, Wi2, bi2 = f32(We), f32(be), f32(Wi1), f32(bi1), f32(Wi2), f32(bi2)

    Bv, Sv, Hv = h.shape
    Ev = Wd2.shape[1]
    TOK = Bv * Sv
    T = TOK // N_CORES
    include_be = bool(np.any(be))

    nc = _get_nc(T, Hv, Ev, include_be)

    hf = h.reshape(TOK, Hv)
    mf = mask.reshape(TOK)
    weights = dict(wd1=Wd1, bd1=bd1, wd2=Wd2, bd2=bd2, we=We, be=be,
                   wi1=Wi1, bi1=bi1, wi2=Wi2, bi2=bi2)
    in_maps = []
    for c in range(N_CORES):
        m = dict(weights)
        m["h"] = np.ascontiguousarray(hf[c * T:(c + 1) * T])
        m["mask"] = np.ascontiguousarray(mf[c * T:(c + 1) * T])
        in_maps.append(m)

    # The first execution of a freshly-loaded NEFF occasionally trips a
    # transient NRT_EXEC_UNIT_UNRECOVERABLE on the axon worker; a retry after a
    # short pause has always succeeded, so tolerate a couple of those.
    last_exc = None
    for attempt in range(3):
        try:
            res = run_bass_kernel_spmd(nc, in_maps,
                                       core_ids=list(range(N_CORES)),
                                       trace=_trace)
            break
        except Exception as e:  # noqa: BLE001 - jax.errors.JaxRuntimeError
            last_exc = e
            if "UNAVAILABLE" not in str(e) and "unrecoverable" not in str(e):
                raise
            import time as _time
            _time.sleep(5 * (attempt + 1))
    else:
        raise last_exc
    out = np.concatenate([res.results[c]["out"] for c in range(N_CORES)], axis=0)
    out = out.reshape(Bv, Sv, Hv).astype(np.float32)
    if _trace:
        kernel._last_results = res
    return out



# revision 4
# speedup vs baseline: 1.0840x; 1.0840x over previous
"""Trainium2 Bass kernel for the EnhancedMathematicalReasoning MoE-routing module.

Computation (per token t, hidden dim H=2048, E=8 experts, dense routing):
    a1     = gelu(h @ Wd1 + bd1)
    logits = a1 @ Wd2 + bd2
    op_w   = softmax(logits)
    comb   = sum_e op_w[:, e] * (h @ We[e] + be[e])
    out    = (gelu(comb @ Wi1 + bi1) @ Wi2 + bi2) * mask

Sharding: data-parallel over the 8192 tokens -> 1024 tokens per NeuronCore,
weights replicated, no collectives.

Per-core strategy (P=128), v2 -- all GEMM operands in bf16:
  - Host casts h + all weights to bf16.  PE streaming rate is identical to
    f32r (1 cycle/row), but bf16 enables FWL fast weight loads (the f32r
    LDWEIGHTS was the exposed +14.5ns/MM tax in v1), halves all DMA traffic
    and makes PE transposes 1.0 cyc/row instead of 1.5.
  - h is PE-transposed once to hT [H, T] (bf16).  GEMM1/experts/GEMM3 run
    with the weight m-chunk stationary and a resident [H,*] activation as a
    512-wide moving operand, accumulating over K=16 chunks in PSUM.
  - Logits are accumulated in [E, T] orientation in a single PSUM bank per
    T-half across all 16 m-chunks (stationary = Wd2 m-slice [128, 8]), so
    softmax needs no transposes: exp on ACT (bias=bd2, no max subtraction --
    logits have sigma ~0.6), partition-sum + reciprocal-broadcast via tiny
    K=8/K=1 matmuls, one DVE multiply -> normalized op_w in [E, T].
  - Expert combine: op_w row e is broadcast to 128 partitions by a K=8
    selector matmul; DVE does comb += psum * ob into a bf16 arena [H, T].
  - GEMM1 runs as two T-half passes (B1/B2) so it can start as soon as the
    first 512 tokens are transposed; the remaining h transposes interleave
    into B1's matmul stream.  Wd1 chunks stay resident (wep bufs=4) so B2
    re-reads them without re-DMA.
  - GEMM4 is flipped: stationary = a2T token-slice [128k, 128t], moving =
    Wi2 column-chunk -> PSUM is directly [T, H]-oriented, the attention
    mask is fused into the eviction as a per-partition ACT scale, and the
    result DMAs straight out.  No output transposes, ~1.5us tail.
"""

import numpy as np
from contextlib import ExitStack

import ml_dtypes

import concourse.bass as bass
import concourse.tile as tile
from concourse import bacc, mybir
from concourse.bass_utils import run_bass_kernel_spmd
from concourse.masks import make_identity

F32 = mybir.dt.float32
BF16 = mybir.dt.bfloat16
AF = mybir.ActivationFunctionType
ALU = mybir.AluOpType
AX = mybir.AxisListType

P = 128
N_CORES = 8

B, S, H_FULL, E_FULL = 4, 2048, 2048, 8


def build_nc(T, H, E, act=AF.Gelu, include_be=False, include_bi2=False):
    """Build + compile the single-core program (same program runs SPMD on all
    cores). T: tokens per core. Requires T % 1024 == 0, H % 512 == 0."""
    assert T % 1024 == 0 and H % 512 == 0 and E <= P
    KT = H // P      # k-chunks of the contraction dim
    TT = T // P      # token 128-blocks
    TB = T // 512    # token 512-blocks
    MT = H // P      # output m-chunks
    C = H // 512     # 512-wide weight column chunks

    nc = bacc.Bacc("TRN2", target_bir_lowering=False, debug=False)

    h_d = nc.dram_tensor("h", [T, H], BF16, kind="ExternalInput").ap()
    msk_d = nc.dram_tensor("mask", [T], F32, kind="ExternalInput").ap()
    wd1_d = nc.dram_tensor("wd1", [H, H], BF16, kind="ExternalInput").ap()
    bd1_d = nc.dram_tensor("bd1", [H], F32, kind="ExternalInput").ap()
    wd2_d = nc.dram_tensor("wd2", [H, E], BF16, kind="ExternalInput").ap()
    bd2_d = nc.dram_tensor("bd2", [E], F32, kind="ExternalInput").ap()
    we_d = nc.dram_tensor("we", [E, H, H], BF16, kind="ExternalInput").ap()
    be_d = nc.dram_tensor("be", [E, H], BF16, kind="ExternalInput").ap()
    wi1_d = nc.dram_tensor("wi1", [H, H], BF16, kind="ExternalInput").ap()
    bi1_d = nc.dram_tensor("bi1", [H], F32, kind="ExternalInput").ap()
    wi2_d = nc.dram_tensor("wi2", [H, H], BF16, kind="ExternalInput").ap()
    bi2_d = nc.dram_tensor("bi2", [H], BF16, kind="ExternalInput").ap()
    out_d = nc.dram_tensor("out", [T, H], F32, kind="ExternalOutput").ap()

    wd1_re = wd1_d.rearrange("(k p) n -> p k n", p=P)
    wi1_re = wi1_d.rearrange("(k p) n -> p k n", p=P)
    wi2_re = wi2_d.rearrange("(k p) n -> p k n", p=P)

    with tile.TileContext(nc) as tc:
        with ExitStack() as ctx:
            const = ctx.enter_context(tc.tile_pool(name="const", bufs=1))
            bigp = ctx.enter_context(tc.tile_pool(name="bigp", bufs=1))
            wep = ctx.enter_context(tc.tile_pool(name="wep", bufs=4))
            hlp = ctx.enter_context(tc.tile_pool(name="hlp", bufs=4))
            a1p = ctx.enter_context(tc.tile_pool(name="a1p", bufs=3))
            tmpp = ctx.enter_context(tc.tile_pool(name="tmpp", bufs=3))
            opb = ctx.enter_context(tc.tile_pool(name="opb", bufs=4))
            osm = ctx.enter_context(tc.tile_pool(name="osm", bufs=4))
            pp = ctx.enter_context(tc.tile_pool(name="pp", bufs=3, space="PSUM"))

            # ---- input DMAs first so they win the queues at kernel start:
            # B1 (GEMM1 over tokens 0..511) needs h tt=0..3 + the first Wd1
            # chunk; everything else trails.
            hls = []
            for tt in range(TT):
                hl = hlp.tile([P, H], BF16, tag="hl", name=f"hl_{tt}")
                nc.sync.dma_start(hl, h_d[tt * P:(tt + 1) * P, :])
                hls.append(hl)
                if tt == TT // 2 - 1:
                    w1tiles = []
                    w1c = wep.tile([P, KT, 512], BF16, tag="we", name="wd1c_0")
                    nc.sync.dma_start(w1c, wd1_re[:, :, 0:512])
                    w1tiles.append(w1c)
            for c in range(1, C):
                w1c = wep.tile([P, KT, 512], BF16, tag="we", name=f"wd1c_{c}")
                nc.sync.dma_start(w1c, wd1_re[:, :, c * 512:(c + 1) * 512])
                w1tiles.append(w1c)

            # ---- engine-generated constants (no DMA) ----
            identF = const.tile([P, P], F32, name="identF")
            make_identity(nc, identF)
            ident = const.tile([P, P], BF16, name="ident")
            nc.scalar.copy(ident, identF)
            ones8 = const.tile([E, 1], BF16, name="ones8")
            nc.vector.memset(ones8, 1.0)
            ones1x8 = const.tile([1, E], BF16, name="ones1x8")
            nc.vector.memset(ones1x8, 1.0)
            ones1xP = const.tile([1, P], BF16, name="ones1xP")
            nc.vector.memset(ones1xP, 1.0)
            # sel8[e', e*128+p] = (e' == e): K=8 selector used to broadcast
            # op_w rows across all 128 partitions via a tiny matmul.
            sel8f = const.tile([E, E, P], F32, name="sel8f")
            nc.gpsimd.memset(sel8f, 0.0)
            nc.gpsimd.affine_select(
                out=sel8f, in_=sel8f, compare_op=ALU.not_equal, fill=1.0,
                base=0, pattern=[[-1, E], [0, P]], channel_multiplier=1)
            sel8 = const.tile([E, E * P], BF16, name="sel8")
            nc.scalar.copy(sel8, sel8f.rearrange("e a p -> e (a p)"))

            # ---- small constant DMAs (after the h/wd1 loads) ----
            wd2_t = const.tile([P, KT, E], BF16, name="wd2_t")
            nc.sync.dma_start(wd2_t, wd2_d.rearrange("(k p) e -> p k e", p=P))
            bd1_t = const.tile([P, KT], F32, name="bd1_t")
            nc.sync.dma_start(bd1_t, bd1_d.rearrange("(k p) -> p k", p=P))
            bi1_t = const.tile([P, KT], F32, name="bi1_t")
            nc.sync.dma_start(bi1_t, bi1_d.rearrange("(k p) -> p k", p=P))
            bd2c = const.tile([E, 1], F32, name="bd2c")
            nc.sync.dma_start(bd2c, bd2_d.unsqueeze(1))
            mask_t = const.tile([P, TT], F32, name="mask_t")
            nc.sync.dma_start(mask_t, msk_d.rearrange("(t p) -> p t", p=P))
            if include_bi2:
                bi2_t = const.tile([1, H], BF16, name="bi2_t")
                nc.sync.dma_start(bi2_t, bi2_d.unsqueeze(0))
            if include_be:
                be_t = const.tile([E, H], BF16, name="be_t")
                nc.sync.dma_start(be_t, be_d)

            expT = const.tile([E, T], BF16, name="expT")
            opwN = const.tile([E, T], BF16, name="opwN")
            recip = const.tile([1, T], BF16, name="recip")

            # ---- batched PE transpose: 4 [128,128] tiles share one PSUM
            # bank, one batched eviction on an alternating engine ----
            ecnt = [0]

            def tbatch(srcs, out3):
                n = len(srcs)
                trp = pp.tile([P, 4, P], BF16, tag="tr", bufs=3, name="trb")
                for i, s in enumerate(srcs):
                    nc.tensor.matmul(trp[:, i, :], s, ident, is_transpose=True,
                                     start=(i == 0), stop=(i == n - 1))
                ecnt[0] += 1
                if ecnt[0] % 2 == 0:
                    nc.scalar.copy(out3, trp[:, :n, :])
                else:
                    nc.vector.tensor_copy(out3, trp[:, :n, :])

            # ---- stage A: transpose h to hT [H, T] (bf16) ----
            hT = bigp.tile([P, KT, T], BF16, tag="A", name="hT")

            def emit_tt_transposes(tt):
                for kg in range(KT // 4):
                    tbatch(
                        [hls[tt][:, (kg * 4 + j) * P:(kg * 4 + j + 1) * P]
                         for j in range(4)],
                        hT[:, kg * 4:kg * 4 + 4, tt * P:(tt + 1) * P])

            for tt in range(TT // 2):
                emit_tt_transposes(tt)
            deferred = [(TT // 2 + i) for i in range(TT - TT // 2)]

            # ---- stage B: a1 = gelu(Wd1.T @ hT + bd1) in [H_out, T], with
            # the logits GEMM accumulated in [E, T] PSUM across all m ----
            lgs = []

            def emit_b_m(tbp, idx, m, c, mi, lg):
                ps = pp.tile([P, 512], F32, tag="mm", bufs=3, name="ps_g1")
                for k in range(KT):
                    nc.tensor.matmul(ps, w1tiles[c][:, k, mi * P:(mi + 1) * P],
                                     hT[:, k, tbp * 512:(tbp + 1) * 512],
                                     start=(k == 0), stop=(k == KT - 1))
                a1 = a1p.tile([P, 512], BF16, tag="a1", name=f"a1_{tbp}_{m}")
                nc.scalar.activation(a1, ps, act, bias=bd1_t[:, m:m + 1])
                nc.tensor.matmul(lg, wd2_t[:, m, :], a1,
                                 start=(idx == 0), stop=(idx == MT - 1))

            # B1: T-half 0.  Interleave the deferred tt transposes (one
            # 4-batch after each m) into the matmul stream.
            lg0 = pp.tile([E, 512], F32, tag="lg", bufs=2, name="lg0")
            lgs.append(lg0)
            dq = [(tt, kg) for tt in deferred for kg in range(KT // 4)]
            for idx in range(MT):
                emit_b_m(0, idx, idx, idx // 4, idx % 4, lg0)
                for _ in range(2):
                    if dq:
                        tt, kg = dq.pop(0)
                        tbatch(
                            [hls[tt][:, (kg * 4 + j) * P:(kg * 4 + j + 1) * P]
                             for j in range(4)],
                            hT[:, kg * 4:kg * 4 + 4, tt * P:(tt + 1) * P])

            # softmax pieces for T-half 0, interleaved into B2 below.
            def emit_softmax_tb(tb, lg):
                # exp(logits + bd2) on ACT: [E, 512]
                nc.scalar.activation(expT[:, tb * 512:(tb + 1) * 512], lg,
                                     AF.Exp, bias=bd2c, scale=1.0)

            def emit_colsum_tb(tb):
                cs = pp.tile([1, 512], F32, tag="lg", bufs=2, name=f"cs{tb}")
                nc.tensor.matmul(cs, ones8,
                                 expT[:, tb * 512:(tb + 1) * 512],
                                 start=True, stop=True)
                return cs

            def emit_recip_tb(tb, cs):
                # bf16 reciprocal: 2^-9 relative error on the softmax
                # normalizer is far inside the kernel's error budget.
                with nc.allow_low_precision(reason="softmax recip in bf16"):
                    nc.vector.reciprocal(recip[:, tb * 512:(tb + 1) * 512], cs)

            def emit_bcast_tb(tb):
                rb = pp.tile([E, 512], F32, tag="tr", bufs=3, name=f"rb{tb}")
                nc.tensor.matmul(rb, ones1x8,
                                 recip[:, tb * 512:(tb + 1) * 512],
                                 start=True, stop=True)
                nc.vector.tensor_tensor(
                    opwN[:, tb * 512:(tb + 1) * 512],
                    expT[:, tb * 512:(tb + 1) * 512], rb, op=ALU.mult)

            def emit_ob(e, tb):
                sp = pp.tile([P, 512], F32, tag="mm", bufs=3, name="sp")
                nc.tensor.matmul(sp, sel8[:, e * P:(e + 1) * P],
                                 opwN[:, tb * 512:(tb + 1) * 512],
                                 start=True, stop=True)
                ob = opb.tile([P, 512], F32, tag="ob", name=f"ob_{e}_{tb}")
                nc.scalar.copy(ob, sp)
                return ob

            # B2: T-half 1, with the T-half-0 softmax chain resolving on
            # ACT/DVE (plus two tiny matmuls) inside the stream.
            lg1 = pp.tile([E, 512], F32, tag="lg", bufs=2, name="lg1")
            lgs.append(lg1)
            sm0 = {}
            for idx in range(MT):
                emit_b_m(1, idx, idx, idx // 4, idx % 4, lg1)
                if idx == 0:
                    emit_softmax_tb(0, lg0)
                elif idx == 1:
                    sm0["cs"] = emit_colsum_tb(0)
                elif idx == 2:
                    emit_recip_tb(0, sm0["cs"])
                elif idx == 3:
                    emit_bcast_tb(0)
                elif idx == 4:
                    sm0["ob"] = emit_ob(0, 0)

            # ---- stage C: expert GEMMs + weighted combine into arena ----
            arena = bigp.tile([P, KT, TT, P], BF16, tag="B", name="arena")

            def emit_expert_psum(e, wec, mi, tb, m, ob):
                ps = pp.tile([P, 512], F32, tag="mm", bufs=3, name="eps")
                for k in range(KT):
                    nc.tensor.matmul(ps, wec[:, k, mi * P:(mi + 1) * P],
                                     hT[:, k, tb * 512:(tb + 1) * 512],
                                     start=(k == 0), stop=(k == KT - 1))
                wsl = arena[:, m, tb * 4:(tb + 1) * 4, :]
                ob3 = ob.rearrange("p (n c) -> p n c", c=P)
                ps3 = ps.rearrange("p (n c) -> p n c", c=P)
                if e == 0 and not include_be:
                    nc.vector.tensor_tensor(wsl, ps3, ob3, op=ALU.mult)
                else:
                    tmp = tmpp.tile([P, 512], F32, tag="t", name="tmp")
                    tmp3 = tmp.rearrange("p (n c) -> p n c", c=P)
                    nc.vector.tensor_tensor(tmp3, ps3, ob3, op=ALU.mult)
                    nc.vector.tensor_tensor(wsl, wsl, tmp3, op=ALU.add)

            def emit_be_init(tb):
                # arena[:, :, tb half] = sum_e op_w[t, e] * be[e, :]
                for m in range(MT):
                    bps = pp.tile([P, 512], F32, tag="mm", bufs=3, name="bps")
                    nc.tensor.matmul(bps, be_t[:, m * P:(m + 1) * P],
                                     opwN[:, tb * 512:(tb + 1) * 512],
                                     start=True, stop=True)
                    nc.scalar.copy(
                        arena[:, m, tb * 4:(tb + 1) * 4, :],
                        bps.rearrange("p (n c) -> p n c", c=P))

            obs = {0: sm0["ob"]}
            wecs = {}
            # Expert 0 runs tb-major: the tb=0 column of psums streams while
            # the T-half-1 softmax chain resolves; ob(0,1) is ready long
            # before the tb=1 column starts.
            sm1 = {"step": 0}

            def tick_sm1():
                s = sm1["step"]
                sm1["step"] += 1
                if s == 0:
                    emit_softmax_tb(1, lg1)
                elif s == 1:
                    sm1["cs"] = emit_colsum_tb(1)
                elif s == 2:
                    emit_recip_tb(1, sm1["cs"])
                elif s == 3:
                    emit_bcast_tb(1)
                elif s == 4:
                    obs[1] = emit_ob(0, 1)

            tick_sm1()
            for tb in range(TB):
                if include_be:
                    emit_be_init(tb)
                for c in range(C):
                    if tb == 0:
                        wec = wep.tile([P, KT, 512], BF16, tag="we",
                                       name=f"we_0_{c}")
                        nc.sync.dma_start(
                            wec,
                            we_d[0].rearrange("(k p) n -> p k n",
                                              p=P)[:, :, c * 512:(c + 1) * 512])
                        wecs[c] = wec
                    for mi in range(4):
                        emit_expert_psum(0, wecs[c], mi, tb, c * 4 + mi,
                                         obs[tb])
                        tick_sm1()

            for e in range(1, E):
                we_re = we_d[e].rearrange("(k p) n -> p k n", p=P)
                for tb in range(TB):
                    obs[tb] = emit_ob(e, tb)
                for c in range(C):
                    wec = wep.tile([P, KT, 512], BF16, tag="we",
                                   name=f"we_{e}_{c}")
                    nc.sync.dma_start(wec,
                                      we_re[:, :, c * 512:(c + 1) * 512])
                    for mi in range(4):
                        for tb in range(TB):
                            emit_expert_psum(e, wec, mi, tb, c * 4 + mi,
                                             obs[tb])

            # ---- stage E: a2T = gelu(Wi1.T @ arena + bi1) [H, T] ----
            a2T = bigp.tile([P, KT, T], BF16, tag="A", name="a2T")
            for c in range(C):
                w3c = wep.tile([P, KT, 512], BF16, tag="we", name=f"wi1c_{c}")
                nc.sync.dma_start(w3c, wi1_re[:, :, c * 512:(c + 1) * 512])
                for mi in range(4):
                    m = c * 4 + mi
                    for tb in range(TB):
                        ps = pp.tile([P, 512], F32, tag="mm", bufs=3,
                                     name="ps_g3")
                        for k in range(KT):
                            nc.tensor.matmul(
                                ps, w3c[:, k, mi * P:(mi + 1) * P],
                                arena[:, k, tb * 4:(tb + 1) * 4, :],
                                start=(k == 0), stop=(k == KT - 1))
                        nc.scalar.activation(
                            a2T[:, m, tb * 512:(tb + 1) * 512], ps, act,
                            bias=bi1_t[:, m:m + 1])

            # ---- stage F (flipped): out[t, n] = a2.T @ Wi2 + bi2, PSUM in
            # [T, H] orientation, mask fused into the eviction, direct DMA ----
            for nb in range(C):
                w4c = wep.tile([P, KT, 512], BF16, tag="we", name=f"wi2c_{nb}")
                nc.sync.dma_start(w4c, wi2_re[:, :, nb * 512:(nb + 1) * 512])
                for tt in range(TT):
                    ps = pp.tile([P, 512], F32, tag="mm", bufs=3, name="ps_g4")
                    if include_bi2:
                        nc.tensor.matmul(ps, ones1xP,
                                         bi2_t[:, nb * 512:(nb + 1) * 512],
                                         start=True, stop=False)
                    for k in range(KT):
                        nc.tensor.matmul(
                            ps, a2T[:, k, tt * P:(tt + 1) * P],
                            w4c[:, k, :],
                            start=(k == 0 and not include_bi2),
                            stop=(k == KT - 1))
                    ot = osm.tile([P, 512], F32, tag="os", name="ot")
                    nc.scalar.activation(ot, ps, AF.Copy,
                                         scale=mask_t[:, tt:tt + 1])
                    nc.sync.dma_start(
                        out_d[tt * P:(tt + 1) * P, nb * 512:(nb + 1) * 512],
                        ot)

    nc.compile()
    return nc


_CACHED = {}


def _get_nc(T, H, E, include_be, include_bi2):
    key = (T, H, E, include_be, include_bi2)
    if key not in _CACHED:
        _CACHED[key] = build_nc(T, H, E, act=AF.Gelu, include_be=include_be,
                                include_bi2=include_bi2)
    return _CACHED[key]


def kernel(hidden_states, attention_mask, Wd1, bd1, Wd2, bd2, We, be, Wi1, bi1,
           Wi2, bi2, _trace=False):
    bf = lambda x: np.ascontiguousarray(
        np.asarray(x, dtype=np.float32).astype(ml_dtypes.bfloat16))
    f32 = lambda x: np.ascontiguousarray(np.asarray(x, dtype=np.float32))
    h = bf(hidden_states)
    mask = f32(attention_mask)
    Wd1b, bd1f, Wd2b, bd2f = bf(Wd1), f32(bd1), bf(Wd2), f32(bd2)
    Web, beb = bf(We), bf(be)
    Wi1b, bi1f, Wi2b, bi2b = bf(Wi1), f32(bi1), bf(Wi2), bf(bi2)

    Bv, Sv, Hv = h.shape
    Ev = Wd2b.shape[1]
    TOK = Bv * Sv
    T = TOK // N_CORES
    include_be = bool(np.any(np.asarray(be)))
    include_bi2 = bool(np.any(np.asarray(bi2)))

    nc = _get_nc(T, Hv, Ev, include_be, include_bi2)

    hf = h.reshape(TOK, Hv)
    mf = mask.reshape(TOK)
    weights = dict(wd1=Wd1b, bd1=bd1f, wd2=Wd2b, bd2=bd2f, we=Web, be=beb,
                   wi1=Wi1b, bi1=bi1f, wi2=Wi2b, bi2=bi2b)
    in_maps = []
    for c in range(N_CORES):
        m = dict(weights)
        m["h"] = np.ascontiguousarray(hf[c * T:(c + 1) * T])
        m["mask"] = np.ascontiguousarray(mf[c * T:(c + 1) * T])
        in_maps.append(m)

    # The first execution of a freshly-loaded NEFF occasionally trips a
    # transient NRT_EXEC_UNIT_UNRECOVERABLE on the axon worker; a retry after a
    # short pause has always succeeded, so tolerate a couple of those.
    last_exc = None
    for attempt in range(3):
        try:
            res = run_bass_kernel_spmd(nc, in_maps,
                                       core_ids=list(range(N_CORES)),
                                       trace=_trace)
            break
        except Exception as e:  # noqa: BLE001 - jax.errors.JaxRuntimeError
            last_exc = e
            if "UNAVAILABLE" not in str(e) and "unrecoverable" not in str(e):
                raise
            import time as _time
            _time.sleep(5 * (attempt + 1))
    else:
        raise last_exc
    out = np.concatenate([res.results[c]["out"] for c in range(N_CORES)], axis=0)
    out = out.reshape(Bv, Sv, Hv).astype(np.float32)
    if _trace:
        kernel._last_results = res
    return out
